# revision 31
# baseline (speedup 1.0000x reference)
"""CrossAttention kernel for 8 TRN2 NeuronCores (head-parallel sharding).

Problem: x[2,2048,1024], context[2,2048,1024], 16 heads x 64 dim,
q/k/v projections + softmax attention + output projection.

Sharding: 2 heads per core (e-slice of 128 rows of Wq/Wk/Wv, 128 cols of Wo).
Each core computes a full-shape partial of the output projection for its
heads; the host sums the 8 partials and adds the bias.

Optimizations vs the fp32r baseline (609us -> ~280us measured):
- all matmuls in bf16: fp32r lowers to fp32_mode=HIGH on HW (~3.5
  cycles/row vs bf16's 1.0 + FWL weight loads); inputs/weights/partial
  outputs DMA'd as bf16 (halves HBM traffic). rel err ~3e-3 vs 2e-2 gate.
- attention software-pipelined: AV lags QK by `lag` key tiles so the PE
  doesn't head-of-line block on the ACT exp stream or on the previous
  call's normalize chain; QK head pairs run concurrently in the PE's
  64x128 row-tiled mode (tile_position (0,0)/(64,0)).
- all non-attention work (batch-1 projections, V transposes, Wo tiles)
  is slotted as filler units inside the 4 attention calls so the PE
  stays warm (HAM duty throttles after ~3.4us idle).
- softmax normalize: exp row-sums folded into the AV matmul via a ones
  column on V^T; reciprocal_approx_fast on the sums (the exact DVE
  reciprocal costs ~7.8us per [1,1024] call). NOTE: custom-DVE ops read
  garbage from PSUM on HW — the sum row must be copied to SBUF first.
- dual DMA-issue queues (gpsimd + sync), per-instruction PSUM limits:
  matmul output <= 1 bank (512 fp32 cols) is a hard ISA constraint
  (walrus NCC_IXCG864 if violated).

Measured plateau ~282us: ~300ns effective per 512-col matmul
(213ns streaming + partially-hidden ~170ns fixed overhead), ACT exp
142us, DMA-issue ramp ~15us head, ~15us tail drain.
"""
import sys

sys.path.insert(0, "/opt/trn_rl_repo")

import numpy as np
from contextlib import ExitStack

import concourse.bass as bass  # noqa: F401
import concourse.tile as tile
from concourse import bacc, mybir
from concourse.bass_utils import run_bass_kernel_spmd
from concourse.masks import make_identity

B, N, M = 2, 2048, 2048
QDIM = 1024
HEADS = 16
DH = 64
INNER = 1024
NCORES = 8
ES = INNER // NCORES        # 128: e-slice (2 heads * 64) per core
SCALE = DH ** -0.5
T = B * N                   # 4096 query tokens; key tokens likewise B*M
KC = QDIM // 128            # 8 contraction chunks for the projections
MT = M // 128               # 16 key tiles per batch
F32 = mybir.dt.float32
BF16 = mybir.dt.bfloat16
EXP = mybir.ActivationFunctionType.Exp


def build_nc(reps: int = 1):
    nc = bacc.Bacc("TRN2", target_bir_lowering=False, debug=False,
                   num_devices=NCORES)
    xT = nc.dram_tensor("xT", [QDIM, T], BF16, kind="ExternalInput").ap()
    cT = nc.dram_tensor("cT", [QDIM, T], BF16, kind="ExternalInput").ap()
    wqT = nc.dram_tensor("wqT", [QDIM, ES], BF16, kind="ExternalInput").ap()
    wkT = nc.dram_tensor("wkT", [QDIM, ES], BF16, kind="ExternalInput").ap()
    wvT = nc.dram_tensor("wvT", [QDIM, ES], BF16, kind="ExternalInput").ap()
    woT = nc.dram_tensor("woT", [ES, QDIM], BF16, kind="ExternalInput").ap()
    part = nc.dram_tensor("part", [T, QDIM], BF16, kind="ExternalOutput").ap()

    xT3 = xT.rearrange("(kc p) n -> kc p n", p=128)
    cT3 = cT.rearrange("(kc p) n -> kc p n", p=128)

    with tile.TileContext(nc) as tc, ExitStack() as ctx:
        const = ctx.enter_context(tc.tile_pool(name="const", bufs=1))
        big = ctx.enter_context(tc.tile_pool(name="bigsb", bufs=1))
        xsl = ctx.enter_context(tc.tile_pool(name="xsl", bufs=18))
        epool = ctx.enter_context(tc.tile_pool(name="epool", bufs=10))
        opool = ctx.enter_context(tc.tile_pool(name="opool", bufs=2))
        bcp = ctx.enter_context(tc.tile_pool(name="bcp", bufs=2))
        outp = ctx.enter_context(tc.tile_pool(name="outp", bufs=3))
        psB = ctx.enter_context(tc.tile_pool(name="psB", bufs=2, space="PSUM"))
        psA = ctx.enter_context(tc.tile_pool(name="psA", bufs=2, space="PSUM"))

        ident = const.tile([128, 128], BF16)
        make_identity(nc, ident[:])
        onesb = const.tile([128, B * MT], BF16)
        nc.vector.memset(onesb[:], 1.0)
        wq_sb = const.tile([128, KC, ES], BF16)
        wk_sb = const.tile([128, KC, ES], BF16)
        wv_sb = const.tile([128, KC, ES], BF16)
        wo_sb = const.tile([128, QDIM], BF16)
        nc.gpsimd.dma_start(wq_sb[:], wqT.rearrange("(kc p) e -> p kc e", p=128))
        nc.gpsimd.dma_start(wk_sb[:], wkT.rearrange("(kc p) e -> p kc e", p=128))
        nc.gpsimd.dma_start(wv_sb[:], wvT.rearrange("(kc p) e -> p kc e", p=128))
        nc.gpsimd.dma_start(wo_sb[:], woT)

        for _rep in range(reps):
            QT = big.tile([128, T], BF16, tag="QT")
            KT = big.tile([128, T], BF16, tag="KT")
            VT = big.tile([128, T], BF16, tag="VT")
            # V^T tiles for both heads: [:, g, h, 0:DH] + ones col at DH
            vgAB = big.tile([128, B * MT, 2, DH + 1], BF16, tag="vgAB")
            nc.vector.memset(vgAB[:, :, :, DH], 1.0)
            ocats = {}

            def load_chunks(src, col0, width=1024, engines=None):
                engines = engines or (nc.gpsimd, nc.sync)
                tiles = []
                for k in range(KC):
                    xs = xsl.tile([128, width], BF16, tag="xs",
                                  padded_shape=[128, 1024])
                    eng = engines[k % len(engines)]
                    eng.dma_start(xs[:], src[k, :, col0:col0 + width])
                    tiles.append(xs)
                return tiles

            def emit_qproj(b, nbp, xs=None):
                """Project 1024 query tokens (both heads) into QT."""
                col0 = b * N + nbp * 1024
                ps = psB.tile([128, 1024], F32, tag="ps")
                if xs is None:
                    xs = load_chunks(xT3, col0)
                for k in range(KC):
                    for h in range(2):
                        sl = slice(h * 512, (h + 1) * 512)
                        nc.tensor.matmul(ps[:, sl], wq_sb[:, k, :], xs[k][:, sl],
                                         start=(k == 0), stop=(k == KC - 1))
                nc.vector.tensor_copy(QT[:, col0:col0 + 1024], ps[:])

            def emit_kvproj(b, nbp, cs=None):
                """Head-phase K+V projection (uses both psum tags)."""
                col0 = b * N + nbp * 1024
                psk = psB.tile([128, 1024], F32, tag="ps")
                psv = psA.tile([128, 1024], F32, tag="pa")
                if cs is None:
                    cs = load_chunks(cT3, col0)
                for k in range(KC):
                    for h in range(2):
                        sl = slice(h * 512, (h + 1) * 512)
                        nc.tensor.matmul(psk[:, sl], wk_sb[:, k, :], cs[k][:, sl],
                                         start=(k == 0), stop=(k == KC - 1))
                        nc.tensor.matmul(psv[:, sl], wv_sb[:, k, :], cs[k][:, sl],
                                         start=(k == 0), stop=(k == KC - 1))
                nc.vector.tensor_copy(KT[:, col0:col0 + 1024], psk[:])
                nc.vector.tensor_copy(VT[:, col0:col0 + 1024], psv[:])

            def emit_proj_half(b, hcol, wsb, dst):
                """Self-contained 512-token single-target projection filler
                (~1.7us of PE work, holds one ps buffer only for its own
                duration — finer units track the per-mc PE stall pattern
                without starving the ACT exp stream)."""
                col0 = b * N + hcol * 512
                ps = psB.tile([128, 512], F32, tag="ps",
                              padded_shape=[128, 1024])
                cs = load_chunks(xT3 if dst is QT else cT3, col0, width=512)
                for k in range(KC):
                    nc.tensor.matmul(ps[:], wsb[:, k, :], cs[k][:],
                                     start=(k == 0), stop=(k == KC - 1))
                nc.vector.tensor_copy(dst[:, col0:col0 + 512], ps[:])

            def proj1_fillers(slot0, step, b, nbp, wsb, dst):
                """Two half-block filler units at consecutive slots."""
                return [(slot0 + i * step,
                         (lambda hc=2 * nbp + i: emit_proj_half(b, hc, wsb, dst)))
                        for i in range(2)]

            def emit_vtr(b, mt0, mt1):
                for mt in range(mt0, mt1):
                    g = b * MT + mt
                    mcol = b * N + mt * 128
                    pt = psB.tile([128, 128], BF16, tag="ps")
                    nc.tensor.transpose(pt[:], VT[0:128, mcol:mcol + 128],
                                        ident[:])
                    nc.vector.tensor_copy(
                        vgAB[:, g, :, 0:DH],
                        pt[:].rearrange("p (h d) -> p h d", d=DH))

            def emit_wo_tile(b, nt, copy_eng="vector"):
                ocat = ocats[b]
                po = psB.tile([128, 1024], F32, tag="ps")
                for ob in range(2):
                    nc.tensor.matmul(po[:, ob * 512:(ob + 1) * 512],
                                     ocat[:, nt * 128:(nt + 1) * 128],
                                     wo_sb[:, ob * 512:(ob + 1) * 512],
                                     start=True, stop=True)
                osb = outp.tile([128, 1024], BF16, tag="os")
                if copy_eng == "scalar":
                    nc.scalar.copy(osb[:], po[:])
                else:
                    nc.vector.tensor_copy(osb[:], po[:])
                nc.sync.dma_start(
                    part[b * N + nt * 128:b * N + (nt + 1) * 128, :], osb[:])

            def emit_attn(b, nhf, fillers=(), lag=4, tail_wo=()):
                """Attention over 1024 query cols (both heads), SW-pipelined:
                AV lags QK by `lag` key tiles so the PE never head-of-line
                blocks on the exp or on the previous call's normalize chain;
                filler units are interleaved to keep the PE warm."""
                fillers = list(fillers)
                if b not in ocats:
                    ocats[b] = opool.tile([128, N], BF16, tag="oc",
                                          name=f"ocat_b{b}_{_rep}")
                ocat = ocats[b]
                qcol = b * N + nhf * 1024
                oA = psA.tile([128, 1024], F32, tag="pa")
                oB = psA.tile([128, 1024], F32, tag="pa")
                es = {}
                fill_at = {}
                for slot, f in fillers:
                    fill_at.setdefault(min(MT - 1, slot), []).append(f)

                def emit_av(mc, head=None):
                    g = b * MT + mc
                    eA, eB = es[mc] if head == 0 else es.pop(mc)
                    last = (mc == MT - 1)
                    pairs = ((0, eA, oA), (1, eB, oB))
                    if head is not None:
                        pairs = (pairs[head],)
                    for h, e, o_ps in pairs:
                        for nb in range(2):
                            sl = slice(nb * 512, (nb + 1) * 512)
                            nc.tensor.matmul(o_ps[0:DH + 1, sl],
                                             vgAB[:, g, h, :],
                                             e[:, sl], start=(mc == 0),
                                             stop=last)

                def emit_norm(o_ps, row0, half):
                    """Normalize one 512-col half (halves the chain latency
                    so dependents can start sooner)."""
                    cl = slice(half * 512, (half + 1) * 512)
                    ssb = bcp.tile([1, 512], F32, tag="ss")
                    nc.scalar.copy(ssb[:], o_ps[DH:DH + 1, cl])
                    rr = bcp.tile([1, 512], F32, tag="rr")
                    nc.vector.reciprocal_approx_fast(rr[:], ssb[:])
                    bc = bcp.tile([DH, 512], F32, tag="bc")
                    nc.gpsimd.partition_broadcast(bc[:], rr[:])
                    nc.vector.tensor_mul(
                        ocat[row0:row0 + DH,
                             nhf * 1024 + half * 512:nhf * 1024 + (half + 1) * 512],
                        o_ps[0:DH, cl], bc[:])

                for mc in range(MT):
                    mcol = b * N + mc * 128
                    stA = psB.tile([128, 1024], F32, tag="ps")
                    stB = psB.tile([128, 1024], F32, tag="ps")
                    for st, row in ((stA, 0), (stB, DH)):
                        for nb in range(2):
                            sl = slice(nb * 512, (nb + 1) * 512)
                            qsl = slice(qcol + nb * 512, qcol + (nb + 1) * 512)
                            nc.tensor.matmul(st[:, sl],
                                             KT[row:row + DH, mcol:mcol + 128],
                                             QT[row:row + DH, qsl],
                                             start=True, stop=True)
                    eA = epool.tile([128, 1024], BF16, tag="e")
                    eB = epool.tile([128, 1024], BF16, tag="e")
                    nc.scalar.activation(eA[:], stA[:], EXP, scale=SCALE)
                    nc.scalar.activation(eB[:], stB[:], EXP, scale=SCALE)
                    es[mc] = (eA, eB)
                    if mc >= lag:
                        emit_av(mc - lag)
                    for f in fill_at.get(mc, ()):
                        f()
                for mc in range(MT - lag, MT - 1):
                    emit_av(mc)
                # final tile: finish head A, start its normalize while the
                # PE runs head B's last AV, then normalize B; `tail_wo`
                # tiles (needing this call's ocat) interleave between the
                # B-half normalizes so they start as early as possible.
                emit_av(MT - 1, head=0)
                emit_norm(oA, 0, 0)
                emit_norm(oA, 0, 1)
                emit_av(MT - 1, head=1)
                emit_norm(oB, DH, 0)
                for nt in tail_wo[:len(tail_wo) // 2]:
                    emit_wo_tile(b, nt, copy_eng="scalar" if nt % 2 else "vector")
                emit_norm(oB, DH, 1)
                for nt in tail_wo[len(tail_wo) // 2:]:
                    emit_wo_tile(b, nt, copy_eng="scalar" if nt % 2 else "vector")

            # ---- emission schedule ----
            # Minimal head phase: only what attn(0,0)'s first tiles need.
            # Everything else becomes slotted filler inside the four
            # attention calls, keeping the PE continuously busy (which also
            # keeps the HAM duty cycle un-throttled).
            # Front-load all head-phase DMAs across FOUR queue engines (DVE
            # and ACT are idle at t=0) so the PE never waits on DMA-issue
            # latency once it starts.
            all_q = (nc.gpsimd, nc.sync, nc.scalar)
            xs_head = load_chunks(xT3, 0, engines=all_q)
            cs_head = load_chunks(cT3, 0, engines=all_q)
            emit_qproj(0, 0, xs=xs_head)
            emit_kvproj(0, 0, cs=cs_head)
            emit_vtr(0, 0, 8)
            # Filler schedule: ~1.7us half-block projection units at explicit
            # slots, sized to tile the per-mc PE stall inside the ACT-paced
            # attention stream. Same-call deps: K halves land before QK needs
            # cols >=1024 (mc 8); V halves + vtr land before AV(g) at g+lag.
            f00 = (proj1_fillers(1, 1, 0, 1, wk_sb, KT)
                   + proj1_fillers(3, 1, 0, 1, wv_sb, VT)
                   + [(5, lambda: emit_vtr(0, 8, 12)),
                      (6, lambda: emit_vtr(0, 12, MT))]
                   + proj1_fillers(8, 1, 0, 1, wq_sb, QT)
                   + proj1_fillers(11, 1, 1, 0, wq_sb, QT))
            f01 = (proj1_fillers(1, 1, 1, 1, wq_sb, QT)
                   + proj1_fillers(4, 1, 1, 0, wk_sb, KT)
                   + proj1_fillers(7, 1, 1, 0, wv_sb, VT)
                   + [(10, lambda: emit_vtr(1, 0, 4)),
                      (11, lambda: emit_vtr(1, 4, 8))]
                   + proj1_fillers(13, 1, 1, 1, wk_sb, KT))
            emit_attn(0, 0, f00)
            emit_attn(0, 1, f01)
            f10 = (proj1_fillers(1, 1, 1, 1, wv_sb, VT)
                   + [(3, lambda: emit_vtr(1, 8, 12)),
                      (4, lambda: emit_vtr(1, 12, MT))]
                   + [(6 + i, (lambda nt=nt: emit_wo_tile(0, nt)))
                      for i, nt in enumerate(range(0, 9))])
            f11 = ([(1 + i, (lambda nt=nt: emit_wo_tile(0, nt)))
                    for i, nt in enumerate(range(9, 16))]
                   + [(8 + i, (lambda nt=nt: emit_wo_tile(1, nt)))
                      for i, nt in enumerate(range(0, 8))])
            emit_attn(1, 0, f10)
            emit_attn(1, 1, f11, tail_wo=list(range(8, 16)))
    nc.compile()
    return nc


def make_in_maps(x, context, Wq, Wk, Wv, Wo):
    bf = mybir.dt.np(BF16)
    x = np.asarray(x, dtype=np.float32)
    context = np.asarray(context, dtype=np.float32)
    Wq = np.asarray(Wq, dtype=np.float32)
    Wk = np.asarray(Wk, dtype=np.float32)
    Wv = np.asarray(Wv, dtype=np.float32)
    Wo = np.asarray(Wo, dtype=np.float32)
    xT = np.ascontiguousarray(x.reshape(T, QDIM).T).astype(bf)
    cT = np.ascontiguousarray(context.reshape(T, QDIM).T).astype(bf)
    in_maps = []
    for c in range(NCORES):
        es = slice(c * ES, (c + 1) * ES)
        in_maps.append({
            "xT": xT,
            "cT": cT,
            "wqT": np.ascontiguousarray(Wq[es, :].T).astype(bf),
            "wkT": np.ascontiguousarray(Wk[es, :].T).astype(bf),
            "wvT": np.ascontiguousarray(Wv[es, :].T).astype(bf),
            "woT": np.ascontiguousarray(Wo[:, es].T).astype(bf),
        })
    return in_maps


_NC_CACHE = {}


def get_nc(reps: int = 1):
    if reps not in _NC_CACHE:
        _NC_CACHE[reps] = build_nc(reps)
    return _NC_CACHE[reps]


def run_on_hw(in_maps, reps: int = 1):
    nc = get_nc(reps)
    return run_bass_kernel_spmd(nc, in_maps, core_ids=list(range(NCORES)))


def kernel(x, context, Wq, Wk, Wv, Wo, bo):
    in_maps = make_in_maps(x, context, Wq, Wk, Wv, Wo)
    res = run_on_hw(in_maps, reps=1)
    acc = res.results[0]["part"].astype(np.float32)
    for i in range(1, NCORES):
        acc += res.results[i]["part"].astype(np.float32)
    acc += np.asarray(bo, dtype=np.float32)[None, :]
    return acc.reshape(B, N, QDIM)
